# revision 1
# baseline (speedup 1.0000x reference)
"""BCJR detector kernel for Trainium2, 8-core batch-parallel.

Layout per core: 128 words on SBUF partitions, 16 trellis states on the
free dim.  Trellis structure (derived from reference._trellis):
  alpha:  a'[st] = (a[st>>1] + a[(st>>1)+8]) * g[st]
  beta:   b'[s]  = (b[2s%16] + b[2s%16+1])  * g[s]
Both gathers are step-0 broadcast access patterns, no real gather needed.
alpha/beta are kept unnormalized with a lazy per-partition rescale every
NORM steps (decisions are invariant to per-(word,t) positive scaling; an
all-underflow row propagates zeros/NaN and decodes to 0 exactly like the
reference's NaN cascade).
"""

import math
import sys

import numpy as np

sys.path.insert(0, "/opt/trn_rl_repo")

B, T, S, MEM, V = 1024, 2048, 16, 4, 4
NCORES = 8
BPC = B // NCORES  # 128 words per core
BLK = 128          # t-steps per g/combine block
NORM = 16          # rescale cadence


def _build(nc, Tn, g_scale, g_bias):
    import concourse.bass as bass  # noqa: F401
    from concourse import mybir, tile
    from concourse.alu_op_type import AluOpType as OP
    from concourse.mybir import ActivationFunctionType as AF

    dt = mybir.dt.float32
    nblk = Tn // BLK

    # packed input: cols [0:Tn]=y, [Tn:Tn+S]=sp
    yin_d = nc.dram_tensor("yin", [BPC, Tn + S], dt, kind="ExternalInput")
    out_d = nc.dram_tensor("dec", [BPC, Tn], dt, kind="ExternalOutput")

    with tile.TileContext(nc) as tc:
        with (
            tc.tile_pool(name="big", bufs=1) as big,
            tc.tile_pool(name="gp", bufs=2) as gp,
            tc.tile_pool(name="sm", bufs=1) as sm,
        ):
            yin_sb = big.tile([BPC, Tn + S], dt, tag="y")
            y_sb = yin_sb[:, 0:Tn]
            sp_sb = yin_sb[:, Tn : Tn + S]
            H = S // 2
            ACH = 1024  # c-store chunk (t-steps) to keep AP offsets small
            cstores = [
                big.tile(
                    [BPC, H * min(ACH, Tn)], dt,
                    name=f"cstore{i}", tag=f"cstore{i}",
                )
                for i in range((Tn + ACH - 1) // ACH)
            ]

            def csl_of(t):
                c = cstores[t // ACH]
                k = t % ACH
                return c[:, k * H : (k + 1) * H]
            bstore = big.tile([BPC, S * BLK], dt, tag="bstore")
            wtile = big.tile([BPC, S * BLK], dt, tag="w")
            dtile = big.tile([BPC, (S // 2) * BLK], dt, tag="dtile")
            upt = sm.tile([BPC, BLK], dt, tag="up")
            dec = sm.tile([BPC, BLK], dt, tag="dec")
            carry = sm.tile([BPC, S], dt, tag="carry")
            c_a = sm.tile([BPC, S], dt, tag="c_a")
            c_b = sm.tile([BPC, S], dt, tag="c_b")
            r_a = sm.tile([BPC, 1], dt, tag="r_a")
            r_b = sm.tile([BPC, 1], dt, tag="r_b")
            s_a = sm.tile([BPC, 1], dt, tag="s_a")
            s_b = sm.tile([BPC, 1], dt, tag="s_b")
            bias_t = sm.tile([BPC, 1], dt, tag="bias")
            nc.vector.memset(bias_t[:, :], float(g_bias))

            nc.sync.dma_start(yin_sb[:, :], yin_d[:, :])

            def gen_g(blk, which):
                """g[:, k*16+s] = exp(scale*(y[t0+k]-sp[s])^2 + bias) for k in blk."""
                g = gp.tile([BPC, S * BLK], dt, tag=f"g{which}")
                t0 = blk * BLK
                yv = (
                    y_sb[:, t0 : t0 + BLK]
                    .unsqueeze(2)
                    .broadcast_to((BPC, BLK, S))
                )
                spv = sp_sb[:, :].unsqueeze(1).broadcast_to((BPC, BLK, S))
                d3 = g[:, :].rearrange("p (k s) -> p k s", s=S)
                nc.gpsimd.tensor_tensor(d3, yv, spv, OP.subtract)
                nc.gpsimd.tensor_tensor(d3, d3, d3, OP.mult)
                nc.scalar.activation(
                    g[:, :], g[:, :], AF.Exp,
                    bias=bias_t[:, :], scale=float(g_scale),
                )
                return g

            # ---------------- alpha pass (forward), pairsum (c) form ------
            # c_t[j] = alpha_t[j] + alpha_t[j+8]  (8 wide); alpha_{t+1} =
            # c_t[s>>1] * g_t[s] materialized transiently in c_a.
            nc.vector.memset(csl_of(0), 0.0)
            nc.vector.memset(cstores[0][:, 0:1], 1.0)
            nc.vector.memset(r_a[:, :], 1.0)
            nc.vector.memset(r_b[:, :], 1.0)
            for blk in range(nblk):
                g = gen_g(blk, "a")
                for k in range(BLK):
                    t = blk * BLK + k
                    if t >= Tn - 1:
                        break
                    cv = (
                        csl_of(t)
                        .unsqueeze(2)
                        .broadcast_to((BPC, 8, 2))
                    )
                    g3 = g[:, k * S : (k + 1) * S].rearrange(
                        "p (a b) -> p a b", b=2
                    )
                    a3 = c_a[:, :].rearrange("p (a b) -> p a b", b=2)
                    if t % NORM == NORM - 1:
                        nc.vector.scalar_tensor_tensor(
                            a3, cv, r_a[:, :], g3, OP.mult, OP.mult,
                            accum_out=s_a[:, :],
                        )
                        nc.vector.reciprocal(r_a[:, :], s_a[:, :])
                    else:
                        nc.vector.tensor_tensor(a3, cv, g3, OP.mult)
                    nc.vector.tensor_tensor(
                        csl_of(t + 1), c_a[:, 0:8], c_a[:, 8:16], OP.add
                    )

            # ---------------- beta pass (backward) + combine ----------------
            for blk in range(nblk - 1, -1, -1):
                g = gen_g(blk, "b")
                for k in range(BLK - 1, -1, -1):
                    t = blk * BLK + k
                    if t == Tn - 1:
                        bprev = None  # init state
                    elif k == BLK - 1:
                        bprev = carry[:, :]
                    else:
                        bprev = bstore[:, (k + 1) * S : (k + 2) * S]
                    bout = bstore[:, k * S : (k + 1) * S]
                    o3 = bout.rearrange("p (a b) -> p a b", a=2)
                    g3 = g[:, k * S : (k + 1) * S].rearrange(
                        "p (a b) -> p a b", a=2
                    )
                    if bprev is None:
                        # b = init [1,0,...,0]; b' [s] = (init[2s%16]+init[2s%16+1])*g
                        # = g[s] if s in {0,8} else 0
                        nc.vector.memset(bout, 0.0)
                        nc.vector.tensor_tensor(
                            bout[:, 0:9:8],
                            g[:, k * S : k * S + 9 : 8],
                            g[:, k * S : k * S + 9 : 8],
                            OP.max,
                        )
                        continue
                    vE = bprev[:, 0:16:2].unsqueeze(1).broadcast_to((BPC, 2, 8))
                    vO = bprev[:, 1:16:2].unsqueeze(1).broadcast_to((BPC, 2, 8))
                    c3 = c_b[:, :].rearrange("p (a b) -> p a b", a=2)
                    nc.vector.tensor_tensor(c3, vE, vO, OP.add)
                    if t % NORM == NORM - 1:
                        nc.vector.scalar_tensor_tensor(
                            o3, c3, r_b[:, :], g3, OP.mult, OP.mult,
                            accum_out=s_b[:, :],
                        )
                        nc.vector.reciprocal(r_b[:, :], s_b[:, :])
                    else:
                        nc.vector.tensor_tensor(o3, c3, g3, OP.mult)
                # save carry for next (lower) block before combine overwrites
                nc.vector.tensor_copy(carry[:, :], bstore[:, 0:S])
                # combine in pairsum form:
                #   up-dn = sum_j c[j] * (w[2j] - w[2j+1]),  w = g*beta
                nc.gpsimd.tensor_tensor(wtile[:, :], g[:, :], bstore[:, :], OP.mult)
                t0 = blk * BLK
                w3 = wtile[:, :].rearrange("p (k s) -> p k s", s=S)
                d3 = dtile[:, :].rearrange("p (k j) -> p k j", j=8)
                nc.gpsimd.tensor_tensor(
                    d3, w3[:, :, 0:16:2], w3[:, :, 1:16:2], OP.subtract
                )
                cch = cstores[t0 // ACH]
                k0 = t0 % ACH
                c3 = cch[:, k0 * H : (k0 + BLK) * H].rearrange(
                    "p (k j) -> p k j", j=8
                )
                nc.gpsimd.tensor_tensor(d3, d3, c3, OP.mult)
                nc.vector.tensor_reduce(
                    upt[:, :], d3, mybir.AxisListType.X, OP.add,
                )
                nc.vector.tensor_scalar(
                    dec[:, :], upt[:, :], 0.0, None, OP.is_lt,
                )
                nc.sync.dma_start(out_d[:, t0 : t0 + BLK], dec[:, :])
    return nc


def _legalize_multiwait(bir):
    """Engine instruction structs embed at most ONE sem wait.  Tile's engine
    queue-depth throttle adds a self-wait to nearly every DVE instruction, so
    instructions that also need a cross-engine wait end up with two and
    walrus rejects them.  Split: move all waits onto a 1-element Memset
    carrier inserted just before (same engine, in-order), leaving the real
    instruction wait-free."""
    n = 0
    for fn in bir["functions"]:
        for blk in fn["blocks"]:
            newl = []
            for inst in blk["instructions"]:
                si = inst.get("sync_info") or {}
                waits = si.get("on_wait") or []
                eng = inst.get("engine")
                if len(waits) >= 2 and eng in (
                    "DVE", "Pool", "Activation", "PE", "SP",
                ):
                    for j, w in enumerate(waits):
                        carrier = {
                            "name": inst["name"] + f"-wc{j}",
                            "opcode": "EventSemaphore",
                            "engine": eng,
                            "ins": [],
                            "outs": [],
                            "sync_info": {"on_wait": [w], "on_update": []},
                        }
                        if "debug" in inst:
                            carrier["debug"] = inst["debug"]
                        newl.append(carrier)
                        n += 1
                    si["on_wait"] = []
                    inst["sync_info"] = si
                newl.append(inst)
            blk["instructions"] = newl
    return n


def _finalize(nc):
    """Apply the multi-wait legalization and pin the serialized BIR."""
    import json as _json

    bir = _json.loads(nc.to_json_bytes())
    _legalize_multiwait(bir)
    bts = _json.dumps(bir).encode()
    nc.to_json_bytes = lambda: bts
    return nc


def _np_f32(x):
    return np.ascontiguousarray(np.asarray(x, dtype=np.float32))


def kernel(y, h, snr):
    import concourse.bass as bass
    from concourse.bass_utils import run_bass_kernel_spmd

    y = _np_f32(y)
    h = _np_f32(h)
    snr_f = float(np.asarray(snr))
    sigma = np.float32(10.0 ** (-snr_f / 10.0))

    bits = (np.arange(S)[:, None] >> np.arange(MEM - 1, -1, -1)) & 1
    syms = (1 - 2 * bits).astype(np.float32)          # [S, MEM]
    sp = (syms @ h[:, ::-1].T).astype(np.float32)     # [S, V]
    sp_b = sp.T[np.arange(BPC) % V].astype(np.float32)  # [BPC, S], same per core

    scale = np.float32(-1.0 / (2.0 * sigma * sigma))
    bias = np.float32(-math.log(math.sqrt(2.0 * math.pi) * sigma))

    nc = bass.Bass()
    _build(nc, T, scale, bias)
    _finalize(nc)

    in_maps = [
        {
            "yin": np.ascontiguousarray(
                np.concatenate([y[c * BPC : (c + 1) * BPC], sp_b], axis=1)
            ),
        }
        for c in range(NCORES)
    ]
    res = run_bass_kernel_spmd(nc, in_maps, core_ids=list(range(NCORES)))
    dec = np.concatenate([r["dec"] for r in res.results], axis=0)  # [B, T]

    out = np.zeros((B, T), np.float32)
    out[:, MEM - 1 :] = dec[:, : T - (MEM - 1)]
    return out



# revision 4
# speedup vs baseline: 2.0355x; 2.0355x over previous
"""BCJR detector kernel v2: quad-step transfer-matrix chains.

Per core: 128 words on partitions.  The 16-state trellis collapses to an
8-dim pairsum recursion c' = C_t c with 2-sparse C_t; the backward
recursion uses E_t = C_t^T exactly, so the beta quad transfer is G4^T.
Chains advance 4 timesteps per slot (one mult + one grouped reduce on
DVE, alpha and beta packed in the same instructions).  Decisions
collapse to  delta[t] = sum_s (-1)^s Ma[t,s] Mb[t,s]  with
Ma[t,s] = g_t[s] c_t[s>>1],  Mb[t,s] = g_t[s] e_{t+1}[s&7]
(index algebra validated in proto.py).  bf16 with lazy joint rescaling:
every deviation from the f32 reference accelerates the underflow death
cascade, never reverses it, so dead (w,t) decode to 0 exactly like the
reference's NaN cascade.
"""

import math
import sys

import numpy as np

sys.path.insert(0, "/opt/trn_rl_repo")

B, T, S, MEM, V = 1024, 2048, 16, 4, 4
NCORES = 8
BPC = B // NCORES   # 128 words per core
Q = T // 4          # 512 quads
NB = 8              # t-blocks
QB = Q // NB        # 64 quads per block
TB = T // NB        # 256 timesteps per block
RESC = 4            # rescale every RESC chain slots


def _ap(base, off, dims):
    """Custom affine AP on base's tensor: free dims = [(stride, count)...]."""
    from concourse.ap import AP

    part = list(base.ap)[0]
    return AP(base.tensor, off, [list(part)] + [list(d) for d in dims])


def _build(nc, Tn, g_scale, g_bias):
    assert Tn == T, "kernel is specialized to T=2048"
    import concourse.bass as bass  # noqa: F401
    from concourse import mybir, tile
    from concourse.alu_op_type import AluOpType as OP
    from concourse.mybir import ActivationFunctionType as AF

    f32 = mybir.dt.float32
    bf16 = mybir.dt.bfloat16
    P = BPC

    yin_d = nc.dram_tensor("yin", [P, T + S], f32, kind="ExternalInput")
    out_d = nc.dram_tensor("dec", [P, T], f32, kind="ExternalOutput")

    lp = nc.allow_low_precision(reason="sign-invariant decode; death-safe bf16")
    lp.__enter__()
    with tile.TileContext(nc) as tc:
        with (
            tc.tile_pool(name="big", bufs=1) as big,
            tc.tile_pool(name="gw", bufs=2) as gwp,
            tc.tile_pool(name="gm", bufs=2) as gmp,
            tc.tile_pool(name="gb", bufs=2) as gbp,
            tc.tile_pool(name="vt", bufs=2) as vtp,
            tc.tile_pool(name="mma", bufs=3) as map_,
            tc.tile_pool(name="mmb", bufs=1) as mbp,
            tc.tile_pool(name="scp", bufs=2) as scp,
        ):
            yin_sb = big.tile([P, T + S], f32, name="yin", tag="yin")
            y_sb = yin_sb[:, 0:T]
            spn = yin_sb[:, T : T + S]          # -sp per (word, state)
            g4a = big.tile([P, Q * 64], bf16, name="g4a", tag="g4a")
            bounds = big.tile([P, (Q + 1) * 16], bf16, name="bounds", tag="bd")
            pt = big.tile([P, QB * 64], bf16, name="pt", tag="pt")
            dsq = big.tile([P, S * TB], bf16, name="dsq", tag="dsq")
            cint = big.tile([P, 3 * 8 * QB], bf16, name="cint", tag="ci")
            eint = big.tile([P, 3 * 8 * QB], bf16, name="eint", tag="ei")
            prods = big.tile([P, 128], bf16, name="prods", tag="pr")
            r_t = big.tile([P, 1], f32, name="r_t", tag="r")
            s_t = big.tile([P, 1], f32, name="s_t", tag="s")
            r2_t = big.tile([P, 1], f32, name="r2_t", tag="r2")
            s2_t = big.tile([P, 1], f32, name="s2_t", tag="s2")
            msk = big.tile([P, 8 * TB], bf16, name="msk", tag="msk")
            dle = big.tile([P, TB], f32, name="dle", tag="dle")
            dlo = big.tile([P, TB], f32, name="dlo", tag="dlo")
            dc_t = big.tile([P, TB], f32, name="dc_t", tag="dc")
            bias_t = big.tile([P, 1], f32, name="bias_t", tag="bias")
            nc.vector.memset(bias_t[:, :], float(g_bias))

            nc.sync.dma_start(yin_sb[:, :], yin_d[:, :])
            nc.vector.memset(bounds[:, 0:16], 0.0)
            nc.vector.memset(bounds[:, Q * 16 : (Q + 1) * 16], 0.0)
            nc.vector.memset(bounds[:, 0:1], 1.0)                      # c_0
            nc.vector.memset(bounds[:, Q * 16 + 8 : Q * 16 + 9], 1.0)  # e_T
            nc.vector.memset(r_t[:, :], 1.0)
            nc.vector.memset(r2_t[:, :], 1.0)
            nc.vector.memset(msk[:, :], 1.0)
            nc.vector.memset(_ap(msk[:, :], 0, [(8, TB), (1, 1)]), 0.0)

            # ---------------- g generation (Act), t-major g[t*16+s] -------
            def gen_g_block(b, pool, nm):
                gt = pool.tile([P, S * TB], bf16, name=f"{nm}{b}", tag="g")
                t0 = b * TB
                for s in range(S):
                    nc.scalar.activation(
                        dsq[:, s * TB : (s + 1) * TB],
                        y_sb[:, t0 : t0 + TB],
                        AF.Square,
                        bias=spn[:, s : s + 1],
                        scale=1.0,
                    )
                # g[s*TB + t] = exp(scale*d + bias), both contiguous
                nc.scalar.activation(
                    gt[:, :], dsq[:, :], AF.Exp,
                    bias=bias_t[:, :], scale=float(g_scale),
                )
                return gt

            # ---------------- V construction (Pool) -----------------------
            # V[u, p, j] = gb[j + 8*(p&1)] * ga[(j>>1) + 4*p],  ga=g[2u],
            # gb=g[2u+1]; storage idx = blockbase + u*32 + p*8 + j.
            def gen_v_block(b, gt, parts=1):
                NU = QB * 2
                vt = vtp.tile([P, NU * 32], bf16, name=f"v{b}", tag="v")
                gv = gt[:, :]
                NP_ = NU // parts
                for pp in range(parts):
                    for p1 in range(2):
                        for p0 in range(2):
                            out = _ap(vt[:, :],
                                      pp * NP_ * 32 + p1 * 16 + p0 * 8,
                                      [(32, NP_), (1, 8)])
                            gbv = _ap(gv, pp * NP_ * 2 + 1 + 8 * TB * p0,
                                      [(2, NP_), (TB, 8)])
                            gav = _ap(gv,
                                      pp * NP_ * 2
                                      + (4 * p0 + 8 * p1) * TB,
                                      [(2, NP_), (TB, 4), (0, 2)])
                            nc.gpsimd.tensor_tensor(out, gbv, gav, OP.mult)
                return vt

            # ---------------- Gboth construction --------------------------
            # Gboth[w]: [0:QB*64] alpha G4A(quad w*QB+u) as [u, j, m];
            # [QB*64:] beta G4T(quad (7-w)*QB+u) as [u, m, j] (transposed
            # content, natural u order; the chain AP reverses u).
            # G4[j,m] = sum_p0 VB[j, 2*(m&1)+p0] * VA[(j>>2)+4*(m&1)+2*p0, m>>1]
            def gen_g4_block(b, vt, qlo=0, qhi=QB):
                NQ = qhi - qlo
                for m0 in range(2):
                    for p0 in range(2):
                        for j2 in range(2):
                            out = _ap(pt[:, :], p0 * QB * 32 + j2 * 16,
                                      [(32, NQ), (4, 4), (1, 4)])
                            vbg = _ap(vt[:, :],
                                      qlo * 64 + 32 + (2 * m0 + p0) * 8
                                      + j2 * 4,
                                      [(64, NQ), (1, 4), (0, 4)])
                            vag = _ap(vt[:, :],
                                      qlo * 64 + 4 * m0 + 2 * p0 + j2,
                                      [(64, NQ), (0, 4), (8, 4)])
                            nc.gpsimd.tensor_tensor(out, vbg, vag, OP.mult)
                    i0 = _ap(pt[:, :], 0, [(32, NQ), (4, 8), (1, 4)])
                    i1 = _ap(pt[:, :], QB * 32, [(32, NQ), (4, 8), (1, 4)])
                    og = _ap(g4a[:, :], (b * QB + qlo) * 64 + m0,
                             [(64, NQ), (8, 8), (2, 4)])
                    nc.gpsimd.tensor_tensor(og, i0, i1, OP.add)

            # ---------------- chain slots (DVE) ---------------------------
            def chain_window(w):
                for uu in range(QB):
                    u = w * QB + uu
                    qb_ = Q - 1 - u
                    resc = u % RESC == RESC - 1
                    # late slots: swap halves so bounds out-stride stays >= 0
                    apos, epos = (u + 1) * 16, qb_ * 16 + 8
                    swap = epos < apos
                    # alpha: prods[da,j,m] = G4A(w)[u][j,m] * c[m]
                    ia0 = _ap(g4a[:, :], (w * QB + uu) * 64, [(8, 8), (1, 8)])
                    ia1 = _ap(bounds[:, :], u * 16, [(0, 8), (1, 8)])
                    oa = _ap(prods[:, :], 64 if swap else 0,
                             [(8, 8), (1, 8)])
                    # beta: prods[db,x=m,y=j] = G4A(7-w)[QB-1-uu][j,m] * e[j]
                    ib0 = _ap(g4a[:, :],
                              ((NB - 1 - w) * QB + (QB - 1 - uu)) * 64,
                              [(1, 8), (8, 8)])
                    ib1 = _ap(bounds[:, :], (qb_ + 1) * 16 + 8,
                              [(0, 8), (1, 8)])
                    ob = _ap(prods[:, :], 0 if swap else 64,
                             [(8, 8), (1, 8)])
                    if resc:
                        nc.vector.scalar_tensor_tensor(
                            oa, ia0, r_t[:, :], ia1, OP.mult, OP.mult,
                            accum_out=s_t[:, :])
                        nc.vector.reciprocal(r_t[:, :], s_t[:, :])
                        nc.vector.scalar_tensor_tensor(
                            ob, ib0, r2_t[:, :], ib1, OP.mult, OP.mult,
                            accum_out=s2_t[:, :])
                        nc.vector.reciprocal(r2_t[:, :], s2_t[:, :])
                    else:
                        nc.vector.tensor_tensor(oa, ia0, ia1, OP.mult)
                        nc.vector.tensor_tensor(ob, ib0, ib1, OP.mult)
                    o3 = _ap(prods[:, :], 0, [(64, 2), (8, 8), (1, 8)])
                    ored = _ap(bounds[:, :], min(apos, epos),
                               [(abs(epos - apos), 2), (1, 8)])
                    nc.vector.tensor_reduce(
                        ored, o3, mybir.AxisListType.X, OP.add
                    )

            # ---------------- interiors + combine -------------------------
            def copy_bounds(b):
                sc = scp.tile([P, 2 * 8 * QB], bf16, name=f"sc{b}", tag="sc")
                nc.vector.tensor_copy(
                    _ap(sc[:, :], 0, [(8, QB), (1, 8)]),
                    _ap(bounds[:, :], b * QB * 16, [(16, QB), (1, 8)]))
                nc.vector.tensor_copy(
                    _ap(sc[:, :], 8 * QB, [(8, QB), (1, 8)]),
                    _ap(bounds[:, :], (b * QB + 1) * 16 + 8,
                        [(16, QB), (1, 8)]))
                return sc

            def m_phase_pool(b, gt, sc):
                """Interior M-products + pairsums + pm (Pool).  Returns the
                tile holding pm = Ma*Mb."""
                q0 = b * QB
                ma = map_.tile([P, TB * S], bf16, name=f"ma{b}", tag="ma")
                mb = mbp.tile([P, TB * S], bf16, name=f"mb{b}", tag="mb")
                gv = gt[:, :]
                for r in range(4):  # alpha, ascending
                    om = _ap(ma[:, :], r * 16,
                             [(64, QB), (2, 8), (1, 2)])
                    gg = _ap(gv, r,
                             [(4, QB), (2 * TB, 8), (TB, 2)])
                    if r == 0:
                        cg = _ap(sc[:, :], 0,
                                 [(8, QB), (1, 8), (0, 2)])
                    else:
                        cg = _ap(cint[:, :], (r - 1) * 8 * QB,
                                 [(8, QB), (1, 8), (0, 2)])
                    nc.gpsimd.tensor_tensor(om, gg, cg, OP.mult)
                    if r < 3:
                        nc.gpsimd.tensor_tensor(
                            _ap(cint[:, :], r * 8 * QB, [(8, QB), (1, 8)]),
                            _ap(ma[:, :], r * 16, [(64, QB), (1, 8)]),
                            _ap(ma[:, :], r * 16 + 8, [(64, QB), (1, 8)]),
                            OP.add,
                        )
                for r in range(3, -1, -1):  # beta, descending
                    om = _ap(mb[:, :], r * 16, [(64, QB), (8, 2), (1, 8)])
                    gg = _ap(gv, r, [(4, QB), (8 * TB, 2), (TB, 8)])
                    if r == 3:
                        eg = _ap(sc[:, :], 8 * QB,
                                 [(8, QB), (0, 2), (1, 8)])
                    else:
                        eg = _ap(eint[:, :], r * 8 * QB,
                                 [(8, QB), (0, 2), (1, 8)])
                    nc.gpsimd.tensor_tensor(om, gg, eg, OP.mult)
                    if r > 0:
                        nc.gpsimd.tensor_tensor(
                            _ap(eint[:, :], (r - 1) * 8 * QB,
                                [(8, QB), (1, 8)]),
                            _ap(mb[:, :], r * 16, [(64, QB), (2, 8)]),
                            _ap(mb[:, :], r * 16 + 1, [(64, QB), (2, 8)]),
                            OP.add,
                        )
                nc.gpsimd.tensor_tensor(ma[:, :], ma[:, :], mb[:, :], OP.mult)
                return ma, mb

            def m_phase_dve(b, ma, mb):
                nc.vector.tensor_reduce(
                    dle[:, :], _ap(ma[:, :], 0, [(16, TB), (2, 8)]),
                    mybir.AxisListType.X, OP.add)
                nc.vector.tensor_reduce(
                    dlo[:, :], _ap(ma[:, :], 1, [(16, TB), (2, 8)]),
                    mybir.AxisListType.X, OP.add)
                nc.vector.tensor_tensor(dle[:, :], dle[:, :], dlo[:, :],
                                        OP.subtract)
                nc.vector.tensor_scalar(dc_t[:, :], dle[:, :], 0.0, None,
                                        OP.is_lt)
                nc.sync.dma_start(out_d[:, b * TB : (b + 1) * TB], dc_t[:, :])

            # ======================= emission ==============================
            pairs = [(0, 7), (1, 6), (2, 5), (3, 4)]
            for k, (a, bb) in enumerate(pairs):
                gta = gen_g_block(a, gwp, "gw")
                vta = gen_v_block(a, gta, parts=4 if k == 0 else 1)
                gtb = gen_g_block(bb, gwp, "gw")
                vtb = gen_v_block(bb, gtb, parts=4 if k == 0 else 1)
                if k == 0:
                    qq = QB // 4
                    # beta needs block 7 tail first; alpha block 0 head first
                    gen_g4_block(a, vta, 0, qq)
                    for i in range(4):
                        gen_g4_block(bb, vtb, QB - (i + 1) * qq, QB - i * qq)
                    for i in range(1, 4):
                        gen_g4_block(a, vta, i * qq, (i + 1) * qq)
                else:
                    gen_g4_block(a, vta)
                    gen_g4_block(bb, vtb)
            pending = []  # (block, pm-tile) awaiting DVE combine
            for w in range(NB):
                chain_window(w)
                for b, ma, mbt in pending:
                    m_phase_dve(b, ma, mbt)
                pending = []
                if w >= 4:
                    for b in (NB - 1 - w, w):
                        sc = copy_bounds(b)
                        gmt = gen_g_block(b, gmp, "gm")
                        ma, mbt = m_phase_pool(b, gmt, sc)
                        if w == NB - 1:
                            m_phase_dve(b, ma, mbt)
                        else:
                            pending.append((b, ma, mbt))
    lp.__exit__(None, None, None)
    return nc


def _legalize_multiwait(bir):
    """Split multi-wait engine instructions (walrus allows one sem wait)."""
    n = 0
    for fn in bir["functions"]:
        for blk in fn["blocks"]:
            newl = []
            for inst in blk["instructions"]:
                si = inst.get("sync_info") or {}
                waits = si.get("on_wait") or []
                eng = inst.get("engine")
                if len(waits) >= 2 and eng in (
                    "DVE", "Pool", "Activation", "PE", "SP",
                ):
                    for j, w in enumerate(waits):
                        carrier = {
                            "name": inst["name"] + f"-wc{j}",
                            "opcode": "EventSemaphore",
                            "engine": eng,
                            "ins": [],
                            "outs": [],
                            "sync_info": {"on_wait": [w], "on_update": []},
                        }
                        if "debug" in inst:
                            carrier["debug"] = inst["debug"]
                        newl.append(carrier)
                        n += 1
                    si["on_wait"] = []
                    inst["sync_info"] = si
                newl.append(inst)
            blk["instructions"] = newl
    return n


def _finalize(nc):
    import json as _json

    bir = _json.loads(nc.to_json_bytes())
    _legalize_multiwait(bir)
    bts = _json.dumps(bir).encode()
    nc.to_json_bytes = lambda: bts
    return nc


def _np_f32(x):
    return np.ascontiguousarray(np.asarray(x, dtype=np.float32))


def _prep(y, h, snr):
    y = _np_f32(y)
    h = _np_f32(h)
    snr_f = float(np.asarray(snr))
    sigma = np.float32(10.0 ** (-snr_f / 10.0))
    bits = (np.arange(S)[:, None] >> np.arange(MEM - 1, -1, -1)) & 1
    syms = (1 - 2 * bits).astype(np.float32)
    sp = (syms @ h[:, ::-1].T).astype(np.float32)           # [S, V]
    spn_b = (-sp.T[np.arange(BPC) % V]).astype(np.float32)  # [BPC, S]
    scale = np.float32(-1.0 / (2.0 * sigma * sigma))
    bias = np.float32(-math.log(math.sqrt(2.0 * math.pi) * sigma))
    return y, spn_b, scale, bias


def kernel(y, h, snr):
    import concourse.bass as bass
    from concourse.bass_utils import run_bass_kernel_spmd

    y, spn_b, scale, bias = _prep(y, h, snr)
    nc = bass.Bass()
    _build(nc, T, scale, bias)
    _finalize(nc)

    in_maps = [
        {
            "yin": np.ascontiguousarray(
                np.concatenate([y[c * BPC : (c + 1) * BPC], spn_b], axis=1)
            ),
        }
        for c in range(NCORES)
    ]
    res = run_bass_kernel_spmd(nc, in_maps, core_ids=list(range(NCORES)))
    dec = np.concatenate([r["dec"] for r in res.results], axis=0)  # [B, T]

    out = np.zeros((B, T), np.float32)
    out[:, MEM - 1 :] = dec[:, : T - (MEM - 1)]
    return out


# revision 5
# speedup vs baseline: 2.0786x; 1.0212x over previous
"""BCJR detector kernel v2: quad-step transfer-matrix chains.

Per core: 128 words on partitions.  The 16-state trellis collapses to an
8-dim pairsum recursion c' = C_t c with 2-sparse C_t; the backward
recursion uses E_t = C_t^T exactly, so the beta quad transfer is G4^T.
Chains advance 4 timesteps per slot (one mult + one grouped reduce on
DVE, alpha and beta packed in the same instructions).  Decisions
collapse to  delta[t] = sum_s (-1)^s Ma[t,s] Mb[t,s]  with
Ma[t,s] = g_t[s] c_t[s>>1],  Mb[t,s] = g_t[s] e_{t+1}[s&7]
(index algebra validated in proto.py).  bf16 with lazy joint rescaling:
every deviation from the f32 reference accelerates the underflow death
cascade, never reverses it, so dead (w,t) decode to 0 exactly like the
reference's NaN cascade.
"""

import math
import sys

import numpy as np

sys.path.insert(0, "/opt/trn_rl_repo")

B, T, S, MEM, V = 1024, 2048, 16, 4, 4
NCORES = 8
BPC = B // NCORES   # 128 words per core
Q = T // 4          # 512 quads
NB = 8              # t-blocks
QB = Q // NB        # 64 quads per block
TB = T // NB        # 256 timesteps per block
RESC = 4            # rescale every RESC chain slots


def _ap(base, off, dims):
    """Custom affine AP on base's tensor: free dims = [(stride, count)...]."""
    from concourse.ap import AP

    part = list(base.ap)[0]
    return AP(base.tensor, off, [list(part)] + [list(d) for d in dims])


def _build(nc, Tn, g_scale, g_bias):
    assert Tn == T, "kernel is specialized to T=2048"
    import concourse.bass as bass  # noqa: F401
    from concourse import mybir, tile
    from concourse.alu_op_type import AluOpType as OP
    from concourse.mybir import ActivationFunctionType as AF

    f32 = mybir.dt.float32
    bf16 = mybir.dt.bfloat16
    P = BPC

    yin_d = nc.dram_tensor("yin", [P, T + S], f32, kind="ExternalInput")
    out_d = nc.dram_tensor("dec", [P, T], f32, kind="ExternalOutput")

    lp = nc.allow_low_precision(reason="sign-invariant decode; death-safe bf16")
    lp.__enter__()
    with tile.TileContext(nc) as tc:
        with (
            tc.tile_pool(name="big", bufs=1) as big,
            tc.tile_pool(name="gw", bufs=2) as gwp,
            tc.tile_pool(name="gm", bufs=2) as gmp,
            tc.tile_pool(name="gb", bufs=2) as gbp,
            tc.tile_pool(name="vt", bufs=2) as vtp,
            tc.tile_pool(name="mma", bufs=3) as map_,
            tc.tile_pool(name="mmb", bufs=1) as mbp,
            tc.tile_pool(name="scp", bufs=2) as scp,
        ):
            yin_sb = big.tile([P, T + S], f32, name="yin", tag="yin")
            y_sb = yin_sb[:, 0:T]
            spn = yin_sb[:, T : T + S]          # -sp per (word, state)
            g4a = big.tile([P, Q * 64], bf16, name="g4a", tag="g4a")
            bounds = big.tile([P, (Q + 1) * 16], bf16, name="bounds", tag="bd")
            pt = big.tile([P, QB * 64], bf16, name="pt", tag="pt")
            dsq = big.tile([P, S * TB], bf16, name="dsq", tag="dsq")
            cint = big.tile([P, 3 * 8 * QB], bf16, name="cint", tag="ci")
            eint = big.tile([P, 3 * 8 * QB], bf16, name="eint", tag="ei")
            prods = big.tile([P, 128], bf16, name="prods", tag="pr")
            rj_t = big.tile([P, 2], f32, name="rj_t", tag="r")
            sj_t = big.tile([P, 2], f32, name="sj_t", tag="s")
            r_t = rj_t[:, 0:1]
            r2_t = rj_t[:, 1:2]
            s_t = sj_t[:, 0:1]
            s2_t = sj_t[:, 1:2]
            msk = big.tile([P, 8 * TB], bf16, name="msk", tag="msk")
            dle = big.tile([P, TB], f32, name="dle", tag="dle")
            dlo = big.tile([P, TB], f32, name="dlo", tag="dlo")
            dc_t = big.tile([P, TB], f32, name="dc_t", tag="dc")
            bias_t = big.tile([P, 1], f32, name="bias_t", tag="bias")
            nc.vector.memset(bias_t, float(g_bias))

            nc.sync.dma_start(yin_sb[:, :], yin_d[:, :])
            nc.vector.memset(bounds[:, 0:16], 0.0)
            nc.vector.memset(bounds[:, Q * 16 : (Q + 1) * 16], 0.0)
            nc.vector.memset(bounds[:, 0:1], 1.0)                      # c_0
            nc.vector.memset(bounds[:, Q * 16 + 8 : Q * 16 + 9], 1.0)  # e_T
            nc.vector.memset(rj_t[:, :], 1.0)
            nc.vector.memset(msk[:, :], 1.0)
            nc.vector.memset(_ap(msk[:, :], 0, [(8, TB), (1, 1)]), 0.0)

            # ---------------- g generation (Act), t-major g[t*16+s] -------
            def gen_g_fill(b, gt, hlo=0, hhi=TB):
                t0 = b * TB + hlo
                HL = hhi - hlo
                for s in range(S):
                    nc.scalar.activation(
                        dsq[:, s * TB + hlo : s * TB + hlo + HL],
                        y_sb[:, t0 : t0 + HL],
                        AF.Square,
                        bias=spn[:, s : s + 1],
                        scale=1.0,
                    )
                # g[s*TB + t] = exp(scale*d + bias)
                nc.scalar.activation(
                    _ap(gt[:, :], hlo, [(TB, S), (1, HL)]),
                    _ap(dsq[:, :], hlo, [(TB, S), (1, HL)]),
                    AF.Exp,
                    bias=bias_t, scale=float(g_scale),
                )

            def gen_g_block(b, pool, nm):
                gt = pool.tile([P, S * TB], bf16, name=f"{nm}{b}", tag="g")
                gen_g_fill(b, gt)
                return gt

            # ---------------- V construction (Pool) -----------------------
            # V[u, p, j] = gb[j + 8*(p&1)] * ga[(j>>1) + 4*p],  ga=g[2u],
            # gb=g[2u+1]; storage idx = blockbase + u*32 + p*8 + j.
            def gen_v_range(vt, gt, ulo, uhi):
                NR = uhi - ulo
                gv = gt[:, :]
                for p1 in range(2):
                    for p0 in range(2):
                        out = _ap(vt[:, :], ulo * 32 + p1 * 16 + p0 * 8,
                                  [(32, NR), (1, 8)])
                        gbv = _ap(gv, ulo * 2 + 1 + 8 * TB * p0,
                                  [(2, NR), (TB, 8)])
                        gav = _ap(gv, ulo * 2 + (4 * p0 + 8 * p1) * TB,
                                  [(2, NR), (TB, 4), (0, 2)])
                        nc.gpsimd.tensor_tensor(out, gbv, gav, OP.mult)

            def gen_v_block(b, gt):
                NU = QB * 2
                vt = vtp.tile([P, NU * 32], bf16, name=f"v{b}", tag="v")
                gen_v_range(vt, gt, 0, NU)
                return vt

            # ---------------- Gboth construction --------------------------
            # Gboth[w]: [0:QB*64] alpha G4A(quad w*QB+u) as [u, j, m];
            # [QB*64:] beta G4T(quad (7-w)*QB+u) as [u, m, j] (transposed
            # content, natural u order; the chain AP reverses u).
            # G4[j,m] = sum_p0 VB[j, 2*(m&1)+p0] * VA[(j>>2)+4*(m&1)+2*p0, m>>1]
            def gen_g4_block(b, vt, qlo=0, qhi=QB):
                NQ = qhi - qlo
                for m0 in range(2):
                    for p0 in range(2):
                        for j2 in range(2):
                            out = _ap(pt[:, :], p0 * QB * 32 + j2 * 16,
                                      [(32, NQ), (4, 4), (1, 4)])
                            vbg = _ap(vt[:, :],
                                      qlo * 64 + 32 + (2 * m0 + p0) * 8
                                      + j2 * 4,
                                      [(64, NQ), (1, 4), (0, 4)])
                            vag = _ap(vt[:, :],
                                      qlo * 64 + 4 * m0 + 2 * p0 + j2,
                                      [(64, NQ), (0, 4), (8, 4)])
                            nc.gpsimd.tensor_tensor(out, vbg, vag, OP.mult)
                    i0 = _ap(pt[:, :], 0, [(32, NQ), (4, 8), (1, 4)])
                    i1 = _ap(pt[:, :], QB * 32, [(32, NQ), (4, 8), (1, 4)])
                    og = _ap(g4a[:, :], (b * QB + qlo) * 64 + m0,
                             [(64, NQ), (8, 8), (2, 4)])
                    nc.gpsimd.tensor_tensor(og, i0, i1, OP.add)

            # ---------------- chain slots (DVE) ---------------------------
            def chain_window(w):
                for uu in range(QB):
                    u = w * QB + uu
                    qb_ = Q - 1 - u
                    resc = u % RESC == RESC - 1
                    # late slots: swap halves so bounds out-stride stays >= 0
                    apos, epos = (u + 1) * 16, qb_ * 16 + 8
                    swap = epos < apos
                    # alpha: prods[da,j,m] = G4A(w)[u][j,m] * c[m]
                    ia0 = _ap(g4a[:, :], (w * QB + uu) * 64, [(8, 8), (1, 8)])
                    ia1 = _ap(bounds[:, :], u * 16, [(0, 8), (1, 8)])
                    oa = _ap(prods[:, :], 64 if swap else 0,
                             [(8, 8), (1, 8)])
                    # beta: prods[db,x=m,y=j] = G4A(7-w)[QB-1-uu][j,m] * e[j]
                    ib0 = _ap(g4a[:, :],
                              ((NB - 1 - w) * QB + (QB - 1 - uu)) * 64,
                              [(1, 8), (8, 8)])
                    ib1 = _ap(bounds[:, :], (qb_ + 1) * 16 + 8,
                              [(0, 8), (1, 8)])
                    ob = _ap(prods[:, :], 0 if swap else 64,
                             [(8, 8), (1, 8)])
                    if resc:
                        nc.vector.scalar_tensor_tensor(
                            oa, ia0, r_t, ia1, OP.mult, OP.mult,
                            accum_out=s_t)
                        nc.vector.scalar_tensor_tensor(
                            ob, ib0, r2_t, ib1, OP.mult, OP.mult,
                            accum_out=s2_t)
                        nc.vector.reciprocal(rj_t[:, :], sj_t[:, :])
                    else:
                        nc.vector.tensor_tensor(oa, ia0, ia1, OP.mult)
                        nc.vector.tensor_tensor(ob, ib0, ib1, OP.mult)
                    o3 = _ap(prods[:, :], 0, [(64, 2), (8, 8), (1, 8)])
                    ored = _ap(bounds[:, :], min(apos, epos),
                               [(abs(epos - apos), 2), (1, 8)])
                    nc.vector.tensor_reduce(
                        ored, o3, mybir.AxisListType.X, OP.add
                    )

            # ---------------- interiors + combine -------------------------
            def copy_bounds(b):
                sc = scp.tile([P, 2 * 8 * QB], bf16, name=f"sc{b}", tag="sc")
                nc.vector.tensor_copy(
                    _ap(sc[:, :], 0, [(8, QB), (1, 8)]),
                    _ap(bounds[:, :], b * QB * 16, [(16, QB), (1, 8)]))
                nc.vector.tensor_copy(
                    _ap(sc[:, :], 8 * QB, [(8, QB), (1, 8)]),
                    _ap(bounds[:, :], (b * QB + 1) * 16 + 8,
                        [(16, QB), (1, 8)]))
                return sc

            def m_phase_pool(b, gt, sc):
                """Interior M-products + pairsums + pm (Pool).  Returns the
                tile holding pm = Ma*Mb."""
                q0 = b * QB
                ma = map_.tile([P, TB * S], bf16, name=f"ma{b}", tag="ma")
                mb = mbp.tile([P, TB * S], bf16, name=f"mb{b}", tag="mb")
                gv = gt[:, :]
                for r in range(4):  # alpha, ascending
                    om = _ap(ma[:, :], r * 16,
                             [(64, QB), (2, 8), (1, 2)])
                    gg = _ap(gv, r,
                             [(4, QB), (2 * TB, 8), (TB, 2)])
                    if r == 0:
                        cg = _ap(sc[:, :], 0,
                                 [(8, QB), (1, 8), (0, 2)])
                    else:
                        cg = _ap(cint[:, :], (r - 1) * 8 * QB,
                                 [(8, QB), (1, 8), (0, 2)])
                    nc.gpsimd.tensor_tensor(om, gg, cg, OP.mult)
                    if r < 3:
                        nc.gpsimd.tensor_tensor(
                            _ap(cint[:, :], r * 8 * QB, [(8, QB), (1, 8)]),
                            _ap(ma[:, :], r * 16, [(64, QB), (1, 8)]),
                            _ap(ma[:, :], r * 16 + 8, [(64, QB), (1, 8)]),
                            OP.add,
                        )
                for r in range(3, -1, -1):  # beta, descending
                    om = _ap(mb[:, :], r * 16, [(64, QB), (8, 2), (1, 8)])
                    gg = _ap(gv, r, [(4, QB), (8 * TB, 2), (TB, 8)])
                    if r == 3:
                        eg = _ap(sc[:, :], 8 * QB,
                                 [(8, QB), (0, 2), (1, 8)])
                    else:
                        eg = _ap(eint[:, :], r * 8 * QB,
                                 [(8, QB), (0, 2), (1, 8)])
                    nc.gpsimd.tensor_tensor(om, gg, eg, OP.mult)
                    if r > 0:
                        nc.gpsimd.tensor_tensor(
                            _ap(eint[:, :], (r - 1) * 8 * QB,
                                [(8, QB), (1, 8)]),
                            _ap(mb[:, :], r * 16, [(64, QB), (2, 8)]),
                            _ap(mb[:, :], r * 16 + 1, [(64, QB), (2, 8)]),
                            OP.add,
                        )
                nc.gpsimd.tensor_tensor(ma[:, :], ma[:, :], mb[:, :], OP.mult)
                return ma, mb

            def m_phase_dve(b, ma, mb):
                # q[t,j] = pm[t,2j] - pm[t,2j+1]  (Pool), then one reduce
                nc.gpsimd.tensor_tensor(
                    _ap(mb[:, :], 0, [(8, TB), (1, 8)]),
                    _ap(ma[:, :], 0, [(16, TB), (2, 8)]),
                    _ap(ma[:, :], 1, [(16, TB), (2, 8)]), OP.subtract)
                nc.vector.tensor_reduce(
                    dle[:, :], _ap(mb[:, :], 0, [(8, TB), (1, 8)]),
                    mybir.AxisListType.X, OP.add)
                nc.vector.tensor_scalar(dc_t[:, :], dle[:, :], 0.0, None,
                                        OP.is_lt)
                nc.sync.dma_start(out_d[:, b * TB : (b + 1) * TB], dc_t[:, :])

            # ======================= emission ==============================
            pairs = [(0, 7), (1, 6), (2, 5), (3, 4)]
            for k, (a, bb) in enumerate(pairs):
                if k == 0:
                    HB, HQ, HU = TB // 2, QB // 2, QB
                    gta = gwp.tile([P, S * TB], bf16, name="gw0", tag="g")
                    gtb = gwp.tile([P, S * TB], bf16, name="gw7", tag="g")
                    vta = vtp.tile([P, QB * 64], bf16, name="v0", tag="v")
                    vtb = vtp.tile([P, QB * 64], bf16, name="v7", tag="v")
                    # alpha head first
                    gen_g_fill(a, gta, 0, HB)
                    gen_v_range(vta, gta, 0, HU)
                    gen_g4_block(a, vta, 0, HQ)
                    # beta tail first
                    gen_g_fill(bb, gtb, HB, TB)
                    gen_v_range(vtb, gtb, HU, 2 * QB)
                    gen_g4_block(bb, vtb, HQ, QB)
                    # remaining halves
                    gen_g_fill(a, gta, HB, TB)
                    gen_v_range(vta, gta, HU, 2 * QB)
                    gen_g4_block(a, vta, HQ, QB)
                    gen_g_fill(bb, gtb, 0, HB)
                    gen_v_range(vtb, gtb, 0, HU)
                    gen_g4_block(bb, vtb, 0, HQ)
                else:
                    gta = gen_g_block(a, gwp, "gw")
                    gen_g4_block(a, gen_v_block(a, gta))
                    gtb = gen_g_block(bb, gwp, "gw")
                    gen_g4_block(bb, gen_v_block(bb, gtb))
            pending = []  # (block, pm-tile) awaiting DVE combine
            for w in range(NB):
                chain_window(w)
                for b, ma, mbt in pending:
                    m_phase_dve(b, ma, mbt)
                pending = []
                if w >= 4:
                    for b in (NB - 1 - w, w):
                        sc = copy_bounds(b)
                        gmt = gen_g_block(b, gmp, "gm")
                        ma, mbt = m_phase_pool(b, gmt, sc)
                        if w == NB - 1:
                            m_phase_dve(b, ma, mbt)
                        else:
                            pending.append((b, ma, mbt))
    lp.__exit__(None, None, None)
    return nc


def _legalize_multiwait(bir):
    """Split multi-wait engine instructions (walrus allows one sem wait)."""
    n = 0
    for fn in bir["functions"]:
        for blk in fn["blocks"]:
            newl = []
            for inst in blk["instructions"]:
                si = inst.get("sync_info") or {}
                waits = si.get("on_wait") or []
                eng = inst.get("engine")
                if len(waits) >= 2 and eng in (
                    "DVE", "Pool", "Activation", "PE", "SP",
                ):
                    for j, w in enumerate(waits):
                        carrier = {
                            "name": inst["name"] + f"-wc{j}",
                            "opcode": "EventSemaphore",
                            "engine": eng,
                            "ins": [],
                            "outs": [],
                            "sync_info": {"on_wait": [w], "on_update": []},
                        }
                        if "debug" in inst:
                            carrier["debug"] = inst["debug"]
                        newl.append(carrier)
                        n += 1
                    si["on_wait"] = []
                    inst["sync_info"] = si
                newl.append(inst)
            blk["instructions"] = newl
    return n


def _finalize(nc):
    import json as _json

    bir = _json.loads(nc.to_json_bytes())
    _legalize_multiwait(bir)
    bts = _json.dumps(bir).encode()
    nc.to_json_bytes = lambda: bts
    return nc


def _np_f32(x):
    return np.ascontiguousarray(np.asarray(x, dtype=np.float32))


def _prep(y, h, snr):
    y = _np_f32(y)
    h = _np_f32(h)
    snr_f = float(np.asarray(snr))
    sigma = np.float32(10.0 ** (-snr_f / 10.0))
    bits = (np.arange(S)[:, None] >> np.arange(MEM - 1, -1, -1)) & 1
    syms = (1 - 2 * bits).astype(np.float32)
    sp = (syms @ h[:, ::-1].T).astype(np.float32)           # [S, V]
    spn_b = (-sp.T[np.arange(BPC) % V]).astype(np.float32)  # [BPC, S]
    scale = np.float32(-1.0 / (2.0 * sigma * sigma))
    bias = np.float32(-math.log(math.sqrt(2.0 * math.pi) * sigma))
    return y, spn_b, scale, bias


def kernel(y, h, snr):
    import concourse.bass as bass
    from concourse.bass_utils import run_bass_kernel_spmd

    y, spn_b, scale, bias = _prep(y, h, snr)
    nc = bass.Bass()
    _build(nc, T, scale, bias)
    _finalize(nc)

    in_maps = [
        {
            "yin": np.ascontiguousarray(
                np.concatenate([y[c * BPC : (c + 1) * BPC], spn_b], axis=1)
            ),
        }
        for c in range(NCORES)
    ]
    res = run_bass_kernel_spmd(nc, in_maps, core_ids=list(range(NCORES)))
    dec = np.concatenate([r["dec"] for r in res.results], axis=0)  # [B, T]

    out = np.zeros((B, T), np.float32)
    out[:, MEM - 1 :] = dec[:, : T - (MEM - 1)]
    return out


# revision 6
# speedup vs baseline: 2.1482x; 1.0335x over previous
"""BCJR detector kernel v2: quad-step transfer-matrix chains.

Per core: 128 words on partitions.  The 16-state trellis collapses to an
8-dim pairsum recursion c' = C_t c with 2-sparse C_t; the backward
recursion uses E_t = C_t^T exactly, so the beta quad transfer is G4^T.
Chains advance 4 timesteps per slot (one mult + one grouped reduce on
DVE, alpha and beta packed in the same instructions).  Decisions
collapse to  delta[t] = sum_s (-1)^s Ma[t,s] Mb[t,s]  with
Ma[t,s] = g_t[s] c_t[s>>1],  Mb[t,s] = g_t[s] e_{t+1}[s&7]
(index algebra validated in proto.py).  bf16 with lazy joint rescaling:
every deviation from the f32 reference accelerates the underflow death
cascade, never reverses it, so dead (w,t) decode to 0 exactly like the
reference's NaN cascade.
"""

import math
import sys

import numpy as np

sys.path.insert(0, "/opt/trn_rl_repo")

B, T, S, MEM, V = 1024, 2048, 16, 4, 4
NCORES = 8
BPC = B // NCORES   # 128 words per core
Q = T // 4          # 512 quads
NB = 8              # t-blocks
QB = Q // NB        # 64 quads per block
TB = T // NB        # 256 timesteps per block
RESC = 4            # rescale every RESC chain slots


def _ap(base, off, dims):
    """Custom affine AP on base's tensor: free dims = [(stride, count)...]."""
    from concourse.ap import AP

    part = list(base.ap)[0]
    return AP(base.tensor, off, [list(part)] + [list(d) for d in dims])


def _build(nc, Tn, g_scale, g_bias):
    assert Tn == T, "kernel is specialized to T=2048"
    import concourse.bass as bass  # noqa: F401
    from concourse import mybir, tile
    from concourse.alu_op_type import AluOpType as OP
    from concourse.mybir import ActivationFunctionType as AF

    f32 = mybir.dt.float32
    bf16 = mybir.dt.bfloat16
    P = BPC

    yin_d = nc.dram_tensor("yin", [P, T + S], f32, kind="ExternalInput")
    out_d = nc.dram_tensor("dec", [P, T], f32, kind="ExternalOutput")

    lp = nc.allow_low_precision(reason="sign-invariant decode; death-safe bf16")
    lp.__enter__()
    with tile.TileContext(nc) as tc:
        with (
            tc.tile_pool(name="big", bufs=1) as big,
            tc.tile_pool(name="gw", bufs=2) as gwp,
            tc.tile_pool(name="gm", bufs=2) as gmp,
            tc.tile_pool(name="gb", bufs=2) as gbp,
            tc.tile_pool(name="vt", bufs=2) as vtp,
            tc.tile_pool(name="mma", bufs=3) as map_,
            tc.tile_pool(name="mmb", bufs=2) as mbp,
            tc.tile_pool(name="scp", bufs=2) as scp,
        ):
            yin_sb = big.tile([P, T + S], f32, name="yin", tag="yin")
            y_sb = yin_sb[:, 0:T]
            spn = yin_sb[:, T : T + S]          # -sp per (word, state)
            g4a = big.tile([P, Q * 64], bf16, name="g4a", tag="g4a")
            bounds = big.tile([P, (Q + 1) * 16], bf16, name="bounds", tag="bd")
            pt = big.tile([P, QB * 64], bf16, name="pt", tag="pt")
            dsq = big.tile([P, S * TB], bf16, name="dsq", tag="dsq")
            cint = big.tile([P, 3 * 8 * QB], bf16, name="cint", tag="ci")
            eint = big.tile([P, 3 * 8 * QB], bf16, name="eint", tag="ei")
            prods = big.tile([P, 128], bf16, name="prods", tag="pr")
            rj_t = big.tile([P, 2], f32, name="rj_t", tag="r")
            sj_t = big.tile([P, 2], f32, name="sj_t", tag="s")
            r_t = rj_t[:, 0:1]
            r2_t = rj_t[:, 1:2]
            s_t = sj_t[:, 0:1]
            s2_t = sj_t[:, 1:2]
            dle = big.tile([P, TB], f32, name="dle", tag="dle")
            dc_t = big.tile([P, TB], f32, name="dc_t", tag="dc")
            bias_t = big.tile([P, 1], f32, name="bias_t", tag="bias")
            nc.vector.memset(bias_t, float(g_bias))

            nc.sync.dma_start(yin_sb[:, :], yin_d[:, :])
            nc.vector.memset(bounds[:, 0:16], 0.0)
            nc.vector.memset(bounds[:, Q * 16 : (Q + 1) * 16], 0.0)
            nc.vector.memset(bounds[:, 0:1], 1.0)                      # c_0
            nc.vector.memset(bounds[:, Q * 16 + 8 : Q * 16 + 9], 1.0)  # e_T
            nc.vector.memset(rj_t[:, :], 1.0)

            # ---------------- g generation (Act), t-major g[t*16+s] -------
            def gen_g_fill(b, gt, hlo=0, hhi=TB):
                t0 = b * TB + hlo
                HL = hhi - hlo
                for s in range(S):
                    nc.scalar.activation(
                        dsq[:, s * TB + hlo : s * TB + hlo + HL],
                        y_sb[:, t0 : t0 + HL],
                        AF.Square,
                        bias=spn[:, s : s + 1],
                        scale=1.0,
                    )
                # g[s*TB + t] = exp(scale*d + bias)
                nc.scalar.activation(
                    _ap(gt[:, :], hlo, [(TB, S), (1, HL)]),
                    _ap(dsq[:, :], hlo, [(TB, S), (1, HL)]),
                    AF.Exp,
                    bias=bias_t, scale=float(g_scale),
                )

            def gen_g_block(b, pool, nm):
                gt = pool.tile([P, S * TB], bf16, name=f"{nm}{b}", tag="g")
                gen_g_fill(b, gt)
                return gt

            # ---------------- V construction (Pool) -----------------------
            # V[u, p, j] = gb[j + 8*(p&1)] * ga[(j>>1) + 4*p],  ga=g[2u],
            # gb=g[2u+1]; storage idx = blockbase + u*32 + p*8 + j.
            def gen_v_range(vt, gt, ulo, uhi):
                NR = uhi - ulo
                gv = gt[:, :]
                for p1 in range(2):
                    for p0 in range(2):
                        out = _ap(vt[:, :], ulo * 32 + p1 * 16 + p0 * 8,
                                  [(32, NR), (1, 8)])
                        gbv = _ap(gv, ulo * 2 + 1 + 8 * TB * p0,
                                  [(2, NR), (TB, 8)])
                        gav = _ap(gv, ulo * 2 + (4 * p0 + 8 * p1) * TB,
                                  [(2, NR), (TB, 4), (0, 2)])
                        nc.gpsimd.tensor_tensor(out, gbv, gav, OP.mult)

            def gen_v_block(b, gt):
                NU = QB * 2
                vt = vtp.tile([P, NU * 32], bf16, name=f"v{b}", tag="v")
                gen_v_range(vt, gt, 0, NU)
                return vt

            # ---------------- Gboth construction --------------------------
            # Gboth[w]: [0:QB*64] alpha G4A(quad w*QB+u) as [u, j, m];
            # [QB*64:] beta G4T(quad (7-w)*QB+u) as [u, m, j] (transposed
            # content, natural u order; the chain AP reverses u).
            # G4[j,m] = sum_p0 VB[j, 2*(m&1)+p0] * VA[(j>>2)+4*(m&1)+2*p0, m>>1]
            def gen_g4_block(b, vt, qlo=0, qhi=QB):
                NQ = qhi - qlo
                for m0 in range(2):
                    for p0 in range(2):
                        for j2 in range(2):
                            out = _ap(pt[:, :], p0 * QB * 32 + j2 * 16,
                                      [(32, NQ), (4, 4), (1, 4)])
                            vbg = _ap(vt[:, :],
                                      qlo * 64 + 32 + (2 * m0 + p0) * 8
                                      + j2 * 4,
                                      [(64, NQ), (1, 4), (0, 4)])
                            vag = _ap(vt[:, :],
                                      qlo * 64 + 4 * m0 + 2 * p0 + j2,
                                      [(64, NQ), (0, 4), (8, 4)])
                            nc.gpsimd.tensor_tensor(out, vbg, vag, OP.mult)
                    i0 = _ap(pt[:, :], 0, [(32, NQ), (4, 8), (1, 4)])
                    i1 = _ap(pt[:, :], QB * 32, [(32, NQ), (4, 8), (1, 4)])
                    og = _ap(g4a[:, :], (b * QB + qlo) * 64 + m0,
                             [(64, NQ), (8, 8), (2, 4)])
                    nc.gpsimd.tensor_tensor(og, i0, i1, OP.add)

            # ---------------- chain slots (DVE) ---------------------------
            def chain_window(w):
                for uu in range(QB):
                    u = w * QB + uu
                    qb_ = Q - 1 - u
                    resc = u % RESC == RESC - 1
                    # late slots: swap halves so bounds out-stride stays >= 0
                    apos, epos = (u + 1) * 16, qb_ * 16 + 8
                    swap = epos < apos
                    # alpha: prods[da,j,m] = G4A(w)[u][j,m] * c[m]
                    ia0 = _ap(g4a[:, :], (w * QB + uu) * 64, [(8, 8), (1, 8)])
                    ia1 = _ap(bounds[:, :], u * 16, [(0, 8), (1, 8)])
                    oa = _ap(prods[:, :], 64 if swap else 0,
                             [(8, 8), (1, 8)])
                    # beta: prods[db,x=m,y=j] = G4A(7-w)[QB-1-uu][j,m] * e[j]
                    ib0 = _ap(g4a[:, :],
                              ((NB - 1 - w) * QB + (QB - 1 - uu)) * 64,
                              [(1, 8), (8, 8)])
                    ib1 = _ap(bounds[:, :], (qb_ + 1) * 16 + 8,
                              [(0, 8), (1, 8)])
                    ob = _ap(prods[:, :], 0 if swap else 64,
                             [(8, 8), (1, 8)])
                    if resc:
                        nc.vector.scalar_tensor_tensor(
                            oa, ia0, r_t, ia1, OP.mult, OP.mult,
                            accum_out=s_t)
                        nc.vector.scalar_tensor_tensor(
                            ob, ib0, r2_t, ib1, OP.mult, OP.mult,
                            accum_out=s2_t)
                        nc.vector.reciprocal(rj_t[:, :], sj_t[:, :])
                    else:
                        nc.vector.tensor_tensor(oa, ia0, ia1, OP.mult)
                        nc.vector.tensor_tensor(ob, ib0, ib1, OP.mult)
                    o3 = _ap(prods[:, :], 0, [(64, 2), (8, 8), (1, 8)])
                    ored = _ap(bounds[:, :], min(apos, epos),
                               [(abs(epos - apos), 2), (1, 8)])
                    nc.vector.tensor_reduce(
                        ored, o3, mybir.AxisListType.X, OP.add
                    )

            # ---------------- interiors + combine -------------------------
            def copy_bounds(b, sc, qlo, qhi):
                NQ = qhi - qlo
                nc.vector.tensor_copy(
                    _ap(sc[:, :], qlo * 8, [(8, NQ), (1, 8)]),
                    _ap(bounds[:, :], (b * QB + qlo) * 16,
                        [(16, NQ), (1, 8)]))
                nc.vector.tensor_copy(
                    _ap(sc[:, :], 8 * QB + qlo * 8, [(8, NQ), (1, 8)]),
                    _ap(bounds[:, :], (b * QB + qlo + 1) * 16 + 8,
                        [(16, NQ), (1, 8)]))

            def m_phase_pool(b, gt, sc, ma, mb, qlo, qhi):
                """Interior M-products + pairsums + pm (Pool), quad range."""
                NQ = qhi - qlo
                gv = gt[:, :]
                for r in range(4):  # alpha, ascending
                    om = _ap(ma[:, :], qlo * 64 + r * 16,
                             [(64, NQ), (2, 8), (1, 2)])
                    gg = _ap(gv, 4 * qlo + r,
                             [(4, NQ), (2 * TB, 8), (TB, 2)])
                    if r == 0:
                        cg = _ap(sc[:, :], qlo * 8,
                                 [(8, NQ), (1, 8), (0, 2)])
                    else:
                        cg = _ap(cint[:, :], (r - 1) * 8 * QB + qlo * 8,
                                 [(8, NQ), (1, 8), (0, 2)])
                    nc.gpsimd.tensor_tensor(om, gg, cg, OP.mult)
                    if r < 3:
                        nc.gpsimd.tensor_tensor(
                            _ap(cint[:, :], r * 8 * QB + qlo * 8,
                                [(8, NQ), (1, 8)]),
                            _ap(ma[:, :], qlo * 64 + r * 16,
                                [(64, NQ), (1, 8)]),
                            _ap(ma[:, :], qlo * 64 + r * 16 + 8,
                                [(64, NQ), (1, 8)]),
                            OP.add,
                        )
                for r in range(3, -1, -1):  # beta, descending
                    om = _ap(mb[:, :], qlo * 64 + r * 16,
                             [(64, NQ), (8, 2), (1, 8)])
                    gg = _ap(gv, 4 * qlo + r,
                             [(4, NQ), (8 * TB, 2), (TB, 8)])
                    if r == 3:
                        eg = _ap(sc[:, :], 8 * QB + qlo * 8,
                                 [(8, NQ), (0, 2), (1, 8)])
                    else:
                        eg = _ap(eint[:, :], r * 8 * QB + qlo * 8,
                                 [(8, NQ), (0, 2), (1, 8)])
                    nc.gpsimd.tensor_tensor(om, gg, eg, OP.mult)
                    if r > 0:
                        nc.gpsimd.tensor_tensor(
                            _ap(eint[:, :], (r - 1) * 8 * QB + qlo * 8,
                                [(8, NQ), (1, 8)]),
                            _ap(mb[:, :], qlo * 64 + r * 16,
                                [(64, NQ), (2, 8)]),
                            _ap(mb[:, :], qlo * 64 + r * 16 + 1,
                                [(64, NQ), (2, 8)]),
                            OP.add,
                        )
                nc.gpsimd.tensor_tensor(
                    _ap(ma[:, :], qlo * 64, [(1, NQ * 64)]),
                    _ap(ma[:, :], qlo * 64, [(1, NQ * 64)]),
                    _ap(mb[:, :], qlo * 64, [(1, NQ * 64)]), OP.mult)

            def m_phase_dve(b, ma, mb, qlo, qhi):
                # q[t,j] = pm[t,2j] - pm[t,2j+1]  (Pool), then one reduce
                NT = (qhi - qlo) * 4
                t0 = qlo * 4
                nc.gpsimd.tensor_tensor(
                    _ap(mb[:, :], t0 * 16, [(8, NT), (1, 8)]),
                    _ap(ma[:, :], t0 * 16, [(16, NT), (2, 8)]),
                    _ap(ma[:, :], t0 * 16 + 1, [(16, NT), (2, 8)]),
                    OP.subtract)
                nc.vector.tensor_reduce(
                    dle[:, t0 : t0 + NT],
                    _ap(mb[:, :], t0 * 16, [(8, NT), (1, 8)]),
                    mybir.AxisListType.X, OP.add)
                nc.vector.tensor_scalar(dc_t[:, t0 : t0 + NT],
                                        dle[:, t0 : t0 + NT], 0.0, None,
                                        OP.is_lt)
                nc.sync.dma_start(out_d[:, b * TB + t0 : b * TB + t0 + NT],
                                  dc_t[:, t0 : t0 + NT])

            # ======================= emission ==============================
            pairs = [(0, 7), (1, 6), (2, 5), (3, 4)]
            for k, (a, bb) in enumerate(pairs):
                if k == 0:
                    HB, HQ, HU = TB // 2, QB // 2, QB
                    gta = gwp.tile([P, S * TB], bf16, name="gw0", tag="g")
                    gtb = gwp.tile([P, S * TB], bf16, name="gw7", tag="g")
                    vta = vtp.tile([P, QB * 64], bf16, name="v0", tag="v")
                    vtb = vtp.tile([P, QB * 64], bf16, name="v7", tag="v")
                    # alpha head first
                    gen_g_fill(a, gta, 0, HB)
                    gen_v_range(vta, gta, 0, HU)
                    gen_g4_block(a, vta, 0, HQ)
                    # beta tail first
                    gen_g_fill(bb, gtb, HB, TB)
                    gen_v_range(vtb, gtb, HU, 2 * QB)
                    gen_g4_block(bb, vtb, HQ, QB)
                    # remaining halves
                    gen_g_fill(a, gta, HB, TB)
                    gen_v_range(vta, gta, HU, 2 * QB)
                    gen_g4_block(a, vta, HQ, QB)
                    gen_g_fill(bb, gtb, 0, HB)
                    gen_v_range(vtb, gtb, 0, HU)
                    gen_g4_block(bb, vtb, 0, HQ)
                else:
                    gta = gen_g_block(a, gwp, "gw")
                    gen_g4_block(a, gen_v_block(a, gta))
                    gtb = gen_g_block(bb, gwp, "gw")
                    gen_g4_block(bb, gen_v_block(bb, gtb))
            pending = []  # (block, ma, mb, qlo, qhi) awaiting DVE combine
            H = QB // 2
            for w in range(NB):
                chain_window(w)
                for ent in pending:
                    m_phase_dve(*ent)
                pending = []
                if w >= 4:
                    blo, bhi = NB - 1 - w, w
                    # allocate per-block scratch
                    tiles = {}
                    for b in (blo, bhi):
                        tiles[b] = (
                            scp.tile([P, 2 * 8 * QB], bf16,
                                     name=f"sc{b}", tag="sc"),
                            gmp.tile([P, S * TB], bf16,
                                     name=f"gm{b}", tag="g"),
                            map_.tile([P, TB * S], bf16,
                                      name=f"ma{b}", tag="ma"),
                            mbp.tile([P, TB * S], bf16,
                                     name=f"mb{b}", tag="mb"),
                        )
                    # ready-first halves: block w low quads (alpha ascends),
                    # block 7-w high quads (beta descends)
                    halves = [(bhi, 0, H), (blo, H, QB),
                              (bhi, H, QB), (blo, 0, H)]
                    for b, qlo, qhi in halves:
                        sc, gmt, ma, mbt = tiles[b]
                        gen_g_fill(b, gmt, qlo * 4, qhi * 4)
                        copy_bounds(b, sc, qlo, qhi)
                        m_phase_pool(b, gmt, sc, ma, mbt, qlo, qhi)
                        if w == NB - 1:
                            m_phase_dve(b, ma, mbt, qlo, qhi)
                        else:
                            pending.append((b, ma, mbt, qlo, qhi))
    lp.__exit__(None, None, None)
    return nc


def _legalize_multiwait(bir):
    """Split multi-wait engine instructions (walrus allows one sem wait)."""
    n = 0
    for fn in bir["functions"]:
        for blk in fn["blocks"]:
            newl = []
            for inst in blk["instructions"]:
                si = inst.get("sync_info") or {}
                waits = si.get("on_wait") or []
                eng = inst.get("engine")
                if len(waits) >= 2 and eng in (
                    "DVE", "Pool", "Activation", "PE", "SP",
                ):
                    for j, w in enumerate(waits):
                        carrier = {
                            "name": inst["name"] + f"-wc{j}",
                            "opcode": "EventSemaphore",
                            "engine": eng,
                            "ins": [],
                            "outs": [],
                            "sync_info": {"on_wait": [w], "on_update": []},
                        }
                        if "debug" in inst:
                            carrier["debug"] = inst["debug"]
                        newl.append(carrier)
                        n += 1
                    si["on_wait"] = []
                    inst["sync_info"] = si
                newl.append(inst)
            blk["instructions"] = newl
    return n


def _finalize(nc):
    import json as _json

    bir = _json.loads(nc.to_json_bytes())
    _legalize_multiwait(bir)
    bts = _json.dumps(bir).encode()
    nc.to_json_bytes = lambda: bts
    return nc


def _np_f32(x):
    return np.ascontiguousarray(np.asarray(x, dtype=np.float32))


def _prep(y, h, snr):
    y = _np_f32(y)
    h = _np_f32(h)
    snr_f = float(np.asarray(snr))
    sigma = np.float32(10.0 ** (-snr_f / 10.0))
    bits = (np.arange(S)[:, None] >> np.arange(MEM - 1, -1, -1)) & 1
    syms = (1 - 2 * bits).astype(np.float32)
    sp = (syms @ h[:, ::-1].T).astype(np.float32)           # [S, V]
    spn_b = (-sp.T[np.arange(BPC) % V]).astype(np.float32)  # [BPC, S]
    scale = np.float32(-1.0 / (2.0 * sigma * sigma))
    bias = np.float32(-math.log(math.sqrt(2.0 * math.pi) * sigma))
    return y, spn_b, scale, bias


def kernel(y, h, snr):
    import concourse.bass as bass
    from concourse.bass_utils import run_bass_kernel_spmd

    y, spn_b, scale, bias = _prep(y, h, snr)
    nc = bass.Bass()
    _build(nc, T, scale, bias)
    _finalize(nc)

    in_maps = [
        {
            "yin": np.ascontiguousarray(
                np.concatenate([y[c * BPC : (c + 1) * BPC], spn_b], axis=1)
            ),
        }
        for c in range(NCORES)
    ]
    res = run_bass_kernel_spmd(nc, in_maps, core_ids=list(range(NCORES)))
    dec = np.concatenate([r["dec"] for r in res.results], axis=0)  # [B, T]

    out = np.zeros((B, T), np.float32)
    out[:, MEM - 1 :] = dec[:, : T - (MEM - 1)]
    return out
